# revision 18
# baseline (speedup 1.0000x reference)
"""Multi-head attention (N=4, L=2048, E=1024, H=16, DK=64) on 8 TRN2 cores.

The reference splits heads with a PLAIN RESHAPE (n, l, H*DK) -> (n, H, l, DK),
so "head" h is really a contiguous block of 128 tokens, and the 2048 attention
positions inside it are (token, s) pairs where s indexes sixteen 64-wide
E-slices.  Equivalently, per (batch, block):
    Qb = q[n, 128b:128b+128, :].reshape(2048, 64)   (same for K, V)
    out_block = softmax(Qb Kb^T / 8) Vb  -> reshape(128, E) -> rows of out
Attention positions are processed in permuted order p' = 128*s + tok (a pure
permutation of the softmax axis, so the result is unchanged after unpermuting).

Sharding: core c owns token rows [n, 256c : 256c+256) for every batch n — two
blocks per batch, eight per core.  Outputs are disjoint rows; the host just
scatters them.  Each core needs the full weights (streamed in halves) and only
its own x columns.

Per-core dataflow (matmuls in float32r, 1 cyc/row at N>=256):
  xTc [E, 1024 tok] resident in SBUF.
  V phase:  V_nat [128 tok, E] per (n, B) via PE, evicted into per-s slices
            [tok, 64] + a ones column -> PV stationary tiles [128, 65].
  Q/K phase: projections emitted as [e_out 128, tok 256] PSUM tiles, evicted
            directly into permuted layout q1t/k1t [128 = 2B x 64d, n, 2048 p'].
  Attention per (n, u=q'-chunk of 256): 16 key tiles, scores for both blocks
            row-packed on the PE (they use disjoint 64-row groups), exp on
            ScalarE over [128, 1024] PSUM groups (scale=1/8 folded),
            PV accumulates [ones|V].T @ expS -> [65, 256] (row 64 = denom),
            1/denom broadcast via a ones-matmul, normalize-evict straight into
            the output-projection operand layout opT [e_in 128, a2, B, tok].
  Out proj per (n, B): accumulate over 8 e_in tiles against woT halves,
            DMA rows out.
"""

import numpy as np

import concourse.bass as bass
import concourse.mybir as mybir
import concourse.tile as tile
from concourse import bacc
from concourse.bass_utils import run_bass_kernel_spmd

N, L, E, H = 4, 2048, 1024, 16
DK = E // H  # 64
NC = 8
BPC = 2  # token blocks per core per batch
TPB = 128  # tokens per block
TPN = BPC * TPB  # 256 tokens per batch per core
TC = N * TPN  # 1024 tokens per core
P = 128
QC = 256  # q' chunk
NQC = 2048 // QC  # 8
NKT = 2048 // P  # 16 key tiles (= s values)
ET = E // P  # 8

F32 = mybir.dt.float32
F32R = mybir.dt.float32r


def build_nc():
    nc = bacc.Bacc("TRN2", target_bir_lowering=False, debug=False, num_devices=NC)

    xTc = nc.dram_tensor("xTc", [E, TC], F32R, kind="ExternalInput").ap()
    wqT = nc.dram_tensor("wqT", [E, E], F32R, kind="ExternalInput").ap()
    wkT = nc.dram_tensor("wkT", [E, E], F32R, kind="ExternalInput").ap()
    wvT = nc.dram_tensor("wvT", [E, E], F32R, kind="ExternalInput").ap()
    woT = nc.dram_tensor("woT", [E, E], F32R, kind="ExternalInput").ap()
    outp = nc.dram_tensor("outp", [TC, E], F32, kind="ExternalOutput").ap()

    with tile.TileContext(nc) as tc:
        with (
            tc.tile_pool(name="const", bufs=1) as const,
            tc.tile_pool(name="wpool", bufs=2) as wpool,
            tc.tile_pool(name="xv", bufs=2) as xv_pool,
            tc.tile_pool(name="qk1", bufs=1) as qk1_pool,
            tc.tile_pool(name="expp", bufs=2) as exp_pool,
            tc.tile_pool(name="opt", bufs=1) as opt_pool,
            tc.tile_pool(name="rec", bufs=2) as rec_pool,
            tc.tile_pool(name="ops", bufs=2) as op_pool,
            tc.tile_pool(name="scps", bufs=2, space="PSUM") as sc_psum,
            tc.tile_pool(name="pvps", bufs=2, space="PSUM") as pv_psum,
            tc.tile_pool(name="auxps", bufs=2, space="PSUM") as aux_psum,
        ):
            ones_f32 = const.tile([P, P], F32)
            nc.vector.memset(ones_f32[:], 1.0)
            ones_row = const.tile([1, DK], F32R)
            nc.vector.tensor_copy(ones_row[:], ones_f32[0:1, 0:DK])

            # ---- resident x ----
            x_sb = xv_pool.tile([P, ET, TC], F32R, tag="xv", name="x_sb")
            nc.sync.dma_start(out=x_sb[:], in_=xTc.rearrange("(a p) t -> p a t", p=P))

            def load_w_half(w_dram, half, nm):
                w_sb = wpool.tile([P, ET, E // 2], F32R, tag="w", name=nm)
                src = w_dram[:, half * (E // 2) : (half + 1) * (E // 2)]
                nc.sync.dma_start(
                    out=w_sb[:], in_=src.rearrange("(a p) d -> p a d", p=P)
                )
                return w_sb

            # ---- V phase: natural orientation, sliced into per-s tiles ----
            v_sb = xv_pool.tile([P, N, BPC, NKT, DK + 1], F32R, tag="xv", name="v_sb")
            for half in range(2):
                wv_sb = load_w_half(wvT, half, f"wv{half}")
                for n in range(N):
                    for B in range(BPC):
                        tok0 = n * TPN + B * TPB
                        ps = aux_psum.tile([P, 512], F32, tag="aux", name="vps")
                        for a in range(ET):
                            nc.tensor.matmul(
                                ps[:],
                                x_sb[:, a, tok0 : tok0 + TPB],
                                wv_sb[:, a, :],
                                start=(a == 0),
                                stop=(a == ET - 1),
                            )
                        for sp in range(8):  # s within this half
                            s = half * 8 + sp
                            nc.vector.tensor_copy(
                                v_sb[:, n, B, s, 0:DK],
                                ps[:, sp * DK : (sp + 1) * DK],
                            )
            # ones column for the softmax denominators
            for n in range(N):
                nc.vector.tensor_copy(
                    v_sb[:, n, :, :, DK], ones_f32[:, 0 : BPC * NKT]
                )

            # ---- Q/K phases: evict straight into permuted q1t/k1t ----
            q1t = qk1_pool.tile([P, N, 2048], F32R, tag="q1", name="q1t")
            k1t = qk1_pool.tile([P, N, 2048], F32R, tag="k1", name="k1t")
            for w_dram, dst, wnm in ((wqT, q1t, "wq"), (wkT, k1t, "wk")):
                for half in range(2):
                    w_sb = load_w_half(w_dram, half, f"{wnm}{half}")
                    for n in range(N):
                        for a2 in range(4):  # e_out tile within half
                            ps = aux_psum.tile([P, QC], F32, tag="aux", name="qkps")
                            for a in range(ET):
                                nc.tensor.matmul(
                                    ps[:],
                                    w_sb[:, a, a2 * P : (a2 + 1) * P],
                                    x_sb[:, a, n * TPN : (n + 1) * TPN],
                                    start=(a == 0),
                                    stop=(a == ET - 1),
                                )
                            for sg in range(2):
                                s = half * 8 + a2 * 2 + sg
                                for B in range(BPC):
                                    nc.vector.tensor_copy(
                                        dst[
                                            B * DK : (B + 1) * DK,
                                            n,
                                            s * TPB : (s + 1) * TPB,
                                        ],
                                        ps[sg * DK : (sg + 1) * DK,
                                           B * TPB : (B + 1) * TPB],
                                    )

            wo_sb = [load_w_half(woT, half, f"wo{half}") for half in range(2)]

            # ---- attention + output projection ----
            for n in range(N):
                opT = opt_pool.tile([P, ET, BPC, TPB], F32R, tag="opT", name="opT")
                for u in range(NQC):
                    qsl = slice(u * QC, (u + 1) * QC)
                    pv = [
                        pv_psum.tile([DK + 1, QC], F32, tag="pv", name=f"pv{_b}")
                        for _b in range(BPC)
                    ]
                    for g in range(NKT // 2):
                        sc = sc_psum.tile([P, BPC, 2, QC], F32, tag="sc")
                        for par in range(2):
                            j = 2 * g + par
                            ksl = slice(j * TPB, (j + 1) * TPB)
                            for B in range(BPC):
                                bsl = slice(B * DK, (B + 1) * DK)
                                nc.tensor.matmul(
                                    sc[:, B, par, :],
                                    k1t[bsl, n, ksl],
                                    q1t[bsl, n, qsl],
                                    start=True,
                                    stop=True,
                                )
                        exps = exp_pool.tile([P, BPC, 2, QC], F32R, tag="exps")
                        nc.scalar.activation(
                            exps[:],
                            sc[:],
                            mybir.ActivationFunctionType.Exp,
                            scale=1.0 / np.sqrt(DK),
                        )
                        for par in range(2):
                            j = 2 * g + par
                            for B in range(BPC):
                                nc.tensor.matmul(
                                    pv[B][:],
                                    v_sb[:, n, B, j, :],
                                    exps[:, B, par, :],
                                    start=(j == 0),
                                    stop=(j == NKT - 1),
                                )
                    for B in range(BPC):
                        rec = rec_pool.tile([1, QC], F32R, tag="rec")
                        with nc.allow_low_precision(reason="softmax denom"):
                            nc.vector.reciprocal(rec[:], pv[B][DK : DK + 1, :])
                        bc = aux_psum.tile([DK, QC], F32, tag="aux", name="bc")
                        nc.tensor.matmul(
                            bc[:], ones_row[:], rec[:], start=True, stop=True
                        )
                        bc_sb = rec_pool.tile([DK, QC], F32, tag="bcs")
                        nc.vector.tensor_copy(bc_sb[:], bc[:])
                        for sg in range(2):
                            # s = 2u + sg -> opT tile a2 = u, partition half sg
                            nc.vector.tensor_mul(
                                opT[sg * DK : (sg + 1) * DK, u, B, :],
                                pv[B][0:DK, sg * TPB : (sg + 1) * TPB],
                                bc_sb[:, sg * TPB : (sg + 1) * TPB],
                            )

                for B in range(BPC):
                    for half in range(2):
                        ps = aux_psum.tile([P, 512], F32, tag="aux", name="opps")
                        for a2 in range(ET):
                            nc.tensor.matmul(
                                ps[:],
                                opT[:, a2, B, :],
                                wo_sb[half][:, a2, :],
                                start=(a2 == 0),
                                stop=(a2 == ET - 1),
                            )
                        op_sb = op_pool.tile([P, 512], F32, tag="op")
                        nc.vector.tensor_copy(op_sb[:], ps[:])
                        r0 = n * TPN + B * TPB
                        nc.sync.dma_start(
                            out=outp[r0 : r0 + TPB, half * 512 : (half + 1) * 512],
                            in_=op_sb[:],
                        )

    nc.compile()
    return nc


_CACHED_NC = None


def get_nc():
    global _CACHED_NC
    if _CACHED_NC is None:
        _CACHED_NC = build_nc()
    return _CACHED_NC


def make_in_maps(inputs):
    x = np.ascontiguousarray(np.asarray(inputs["x"], dtype=np.float32))
    Wq = np.asarray(inputs["Wq"], dtype=np.float32)
    Wk = np.asarray(inputs["Wk"], dtype=np.float32)
    Wv = np.asarray(inputs["Wv"], dtype=np.float32)
    Wo = np.asarray(inputs["Wo"], dtype=np.float32)

    wqT = np.ascontiguousarray(Wq.T)
    wkT = np.ascontiguousarray(Wk.T)
    wvT = np.ascontiguousarray(Wv.T)
    woT = np.ascontiguousarray(Wo.T)
    xr = x.reshape(N, L, E)

    in_maps = []
    for c in range(NC):
        # tokens [n, 256c : 256c+256) for each n, transposed to (E, 1024)
        xc = np.concatenate(
            [xr[n, 256 * c : 256 * (c + 1), :] for n in range(N)], axis=0
        )
        in_maps.append(
            {
                "xTc": np.ascontiguousarray(xc.T),
                "wqT": wqT,
                "wkT": wkT,
                "wvT": wvT,
                "woT": woT,
            }
        )
    return in_maps


def kernel(x, Wq, Wk, Wv, Wo):
    in_maps = make_in_maps({"x": x, "Wq": Wq, "Wk": Wk, "Wv": Wv, "Wo": Wo})
    res = run_bass_kernel_spmd(get_nc(), in_maps, list(range(NC)))
    out = np.empty((N, L, E), dtype=np.float32)
    for c in range(NC):
        o = res.results[c]["outp"].reshape(N, TPN, E)
        out[:, 256 * c : 256 * (c + 1), :] = o
    return out


# revision 19
# speedup vs baseline: 1.1340x; 1.1340x over previous
"""Multi-head attention (N=4, L=2048, E=1024, H=16, DK=64) on 8 TRN2 cores.

The reference splits heads with a PLAIN RESHAPE (n, l, H*DK) -> (n, H, l, DK),
so "head" h is really a contiguous block of 128 tokens, and the 2048 attention
positions inside it are (token, s) pairs where s indexes sixteen 64-wide
E-slices.  Equivalently, per (batch, block):
    Qb = q[n, 128b:128b+128, :].reshape(2048, 64)   (same for K, V)
    out_block = softmax(Qb Kb^T / 8) Vb  -> reshape(128, E) -> rows of out
Attention positions are processed in permuted order p' = 128*s + tok (a pure
permutation of the softmax axis, so the result is unchanged after unpermuting).

Sharding: core c owns token rows [n, 256c : 256c+256) for every batch n — two
blocks per batch, eight per core.  Outputs are disjoint rows; the host just
scatters them.  Each core needs the full weights (streamed in halves) and only
its own x columns.

Per-core dataflow (matmuls in float32r, 1 cyc/row at N>=256):
  xTc [E, 1024 tok] resident in SBUF.
  V phase:  V_nat [128 tok, E] per (n, B) via PE, evicted into per-s slices
            [tok, 64] + a ones column -> PV stationary tiles [128, 65].
  Q/K phase: projections emitted as [e_out 128, tok 256] PSUM tiles, evicted
            directly into permuted layout q1t/k1t [128 = 2B x 64d, n, 2048 p'].
  Attention per (n, u=q'-chunk of 256): 16 key tiles, scores for both blocks
            row-packed on the PE (they use disjoint 64-row groups), exp on
            ScalarE over [128, 1024] PSUM groups (scale=1/8 folded),
            PV accumulates [ones|V].T @ expS -> [65, 256] (row 64 = denom),
            1/denom broadcast via a ones-matmul, normalize-evict straight into
            the output-projection operand layout opT [e_in 128, a2, B, tok].
  Out proj per (n, B): accumulate over 8 e_in tiles against woT halves,
            DMA rows out.
"""

import ml_dtypes
import numpy as np

import concourse.bass as bass
import concourse.mybir as mybir
import concourse.tile as tile
from concourse import bacc
from concourse.bass_utils import run_bass_kernel_spmd

N, L, E, H = 4, 2048, 1024, 16
DK = E // H  # 64
NC = 8
BPC = 2  # token blocks per core per batch
TPB = 128  # tokens per block
TPN = BPC * TPB  # 256 tokens per batch per core
TC = N * TPN  # 1024 tokens per core
P = 128
QC = 256  # q' chunk
NQC = 2048 // QC  # 8
NKT = 2048 // P  # 16 key tiles (= s values)
ET = E // P  # 8

F32 = mybir.dt.float32
F32R = mybir.dt.float32r
BF16 = mybir.dt.bfloat16
MM_DT = BF16  # dtype for the bulk matmuls; F32R fallback is more accurate, slower


def build_nc():
    nc = bacc.Bacc("TRN2", target_bir_lowering=False, debug=False, num_devices=NC)

    xTc = nc.dram_tensor("xTc", [E, TC], MM_DT, kind="ExternalInput").ap()
    wqT = nc.dram_tensor("wqT", [E, E], MM_DT, kind="ExternalInput").ap()
    wkT = nc.dram_tensor("wkT", [E, E], MM_DT, kind="ExternalInput").ap()
    wvT = nc.dram_tensor("wvT", [E, E], MM_DT, kind="ExternalInput").ap()
    woT = nc.dram_tensor("woT", [E, E], MM_DT, kind="ExternalInput").ap()
    outp = nc.dram_tensor("outp", [TC, E], F32, kind="ExternalOutput").ap()

    with tile.TileContext(nc) as tc:
        with (
            tc.tile_pool(name="const", bufs=1) as const,
            tc.tile_pool(name="wpool", bufs=2) as wpool,
            tc.tile_pool(name="xv", bufs=2) as xv_pool,
            tc.tile_pool(name="qk1", bufs=1) as qk1_pool,
            tc.tile_pool(name="expp", bufs=2) as exp_pool,
            tc.tile_pool(name="opt", bufs=1) as opt_pool,
            tc.tile_pool(name="rec", bufs=2) as rec_pool,
            tc.tile_pool(name="ops", bufs=2) as op_pool,
            tc.tile_pool(name="scps", bufs=2, space="PSUM") as sc_psum,
            tc.tile_pool(name="pvps", bufs=2, space="PSUM") as pv_psum,
            tc.tile_pool(name="auxps", bufs=2, space="PSUM") as aux_psum,
        ):
            ones_f32 = const.tile([P, P], F32)
            nc.vector.memset(ones_f32[:], 1.0)
            ones_row = const.tile([1, DK], F32R)
            nc.vector.tensor_copy(ones_row[:], ones_f32[0:1, 0:DK])

            # ---- resident x ----
            x_sb = xv_pool.tile([P, ET, TC], MM_DT, tag="xv", name="x_sb")
            nc.sync.dma_start(out=x_sb[:], in_=xTc.rearrange("(a p) t -> p a t", p=P))

            def load_w_half(w_dram, half, nm):
                w_sb = wpool.tile([P, ET, E // 2], MM_DT, tag="w", name=nm)
                src = w_dram[:, half * (E // 2) : (half + 1) * (E // 2)]
                nc.sync.dma_start(
                    out=w_sb[:], in_=src.rearrange("(a p) d -> p a d", p=P)
                )
                return w_sb

            # ---- V phase: natural orientation, sliced into per-s tiles ----
            v_sb = xv_pool.tile([P, N, BPC, NKT, DK + 1], MM_DT, tag="xv", name="v_sb")
            for half in range(2):
                wv_sb = load_w_half(wvT, half, f"wv{half}")
                for n in range(N):
                    for B in range(BPC):
                        tok0 = n * TPN + B * TPB
                        ps = aux_psum.tile([P, 512], F32, tag="aux", name="vps")
                        for a in range(ET):
                            nc.tensor.matmul(
                                ps[:],
                                x_sb[:, a, tok0 : tok0 + TPB],
                                wv_sb[:, a, :],
                                start=(a == 0),
                                stop=(a == ET - 1),
                            )
                        for sp in range(8):  # s within this half
                            s = half * 8 + sp
                            nc.vector.tensor_copy(
                                v_sb[:, n, B, s, 0:DK],
                                ps[:, sp * DK : (sp + 1) * DK],
                            )
            # ones column for the softmax denominators
            for n in range(N):
                nc.vector.tensor_copy(
                    v_sb[:, n, :, :, DK], ones_f32[:, 0 : BPC * NKT]
                )

            # ---- Q/K phases: evict straight into permuted q1t/k1t ----
            q1t = qk1_pool.tile([P, N, 2048], MM_DT, tag="q1", name="q1t")
            k1t = qk1_pool.tile([P, N, 2048], MM_DT, tag="k1", name="k1t")
            for w_dram, dst, wnm in ((wqT, q1t, "wq"), (wkT, k1t, "wk")):
                for half in range(2):
                    w_sb = load_w_half(w_dram, half, f"{wnm}{half}")
                    for n in range(N):
                        for a2 in range(4):  # e_out tile within half
                            ps = aux_psum.tile([P, QC], F32, tag="aux", name="qkps")
                            for a in range(ET):
                                nc.tensor.matmul(
                                    ps[:],
                                    w_sb[:, a, a2 * P : (a2 + 1) * P],
                                    x_sb[:, a, n * TPN : (n + 1) * TPN],
                                    start=(a == 0),
                                    stop=(a == ET - 1),
                                )
                            for sg in range(2):
                                s = half * 8 + a2 * 2 + sg
                                for B in range(BPC):
                                    nc.vector.tensor_copy(
                                        dst[
                                            B * DK : (B + 1) * DK,
                                            n,
                                            s * TPB : (s + 1) * TPB,
                                        ],
                                        ps[sg * DK : (sg + 1) * DK,
                                           B * TPB : (B + 1) * TPB],
                                    )

            wo_sb = [load_w_half(woT, half, f"wo{half}") for half in range(2)]

            # ---- attention + output projection ----
            for n in range(N):
                opT = opt_pool.tile([P, ET, BPC, TPB], MM_DT, tag="opT", name="opT")
                for u in range(NQC):
                    qsl = slice(u * QC, (u + 1) * QC)
                    pv = [
                        pv_psum.tile([DK + 1, QC], F32, tag="pv", name=f"pv{_b}")
                        for _b in range(BPC)
                    ]
                    for g in range(NKT // 2):
                        sc = sc_psum.tile([P, BPC, 2, QC], F32, tag="sc")
                        for par in range(2):
                            j = 2 * g + par
                            ksl = slice(j * TPB, (j + 1) * TPB)
                            for B in range(BPC):
                                bsl = slice(B * DK, (B + 1) * DK)
                                nc.tensor.matmul(
                                    sc[:, B, par, :],
                                    k1t[bsl, n, ksl],
                                    q1t[bsl, n, qsl],
                                    start=True,
                                    stop=True,
                                )
                        exps = exp_pool.tile([P, BPC, 2, QC], MM_DT, tag="exps")
                        nc.scalar.activation(
                            exps[:],
                            sc[:],
                            mybir.ActivationFunctionType.Exp,
                            scale=1.0 / np.sqrt(DK),
                        )
                        for par in range(2):
                            j = 2 * g + par
                            for B in range(BPC):
                                nc.tensor.matmul(
                                    pv[B][:],
                                    v_sb[:, n, B, j, :],
                                    exps[:, B, par, :],
                                    start=(j == 0),
                                    stop=(j == NKT - 1),
                                )
                    for B in range(BPC):
                        rec = rec_pool.tile([1, QC], F32R, tag="rec")
                        with nc.allow_low_precision(reason="softmax denom"):
                            nc.vector.reciprocal(rec[:], pv[B][DK : DK + 1, :])
                        bc = aux_psum.tile([DK, QC], F32, tag="aux", name="bc")
                        nc.tensor.matmul(
                            bc[:], ones_row[:], rec[:], start=True, stop=True
                        )
                        bc_sb = rec_pool.tile([DK, QC], F32, tag="bcs")
                        nc.vector.tensor_copy(bc_sb[:], bc[:])
                        for sg in range(2):
                            # s = 2u + sg -> opT tile a2 = u, partition half sg
                            nc.vector.tensor_mul(
                                opT[sg * DK : (sg + 1) * DK, u, B, :],
                                pv[B][0:DK, sg * TPB : (sg + 1) * TPB],
                                bc_sb[:, sg * TPB : (sg + 1) * TPB],
                            )

                for B in range(BPC):
                    for half in range(2):
                        ps = aux_psum.tile([P, 512], F32, tag="aux", name="opps")
                        for a2 in range(ET):
                            nc.tensor.matmul(
                                ps[:],
                                opT[:, a2, B, :],
                                wo_sb[half][:, a2, :],
                                start=(a2 == 0),
                                stop=(a2 == ET - 1),
                            )
                        op_sb = op_pool.tile([P, 512], F32, tag="op")
                        nc.vector.tensor_copy(op_sb[:], ps[:])
                        r0 = n * TPN + B * TPB
                        nc.sync.dma_start(
                            out=outp[r0 : r0 + TPB, half * 512 : (half + 1) * 512],
                            in_=op_sb[:],
                        )

    nc.compile()
    return nc


_CACHED_NC = None


def get_nc():
    global _CACHED_NC
    if _CACHED_NC is None:
        _CACHED_NC = build_nc()
    return _CACHED_NC


def make_in_maps(inputs):
    x = np.ascontiguousarray(np.asarray(inputs["x"], dtype=np.float32))
    Wq = np.asarray(inputs["Wq"], dtype=np.float32)
    Wk = np.asarray(inputs["Wk"], dtype=np.float32)
    Wv = np.asarray(inputs["Wv"], dtype=np.float32)
    Wo = np.asarray(inputs["Wo"], dtype=np.float32)

    cast = (lambda a: np.ascontiguousarray(a).astype(ml_dtypes.bfloat16)) \
        if MM_DT == BF16 else np.ascontiguousarray
    wqT = cast(Wq.T)
    wkT = cast(Wk.T)
    wvT = cast(Wv.T)
    woT = cast(Wo.T)
    xr = x.reshape(N, L, E)

    in_maps = []
    for c in range(NC):
        # tokens [n, 256c : 256c+256) for each n, transposed to (E, 1024)
        xc = np.concatenate(
            [xr[n, 256 * c : 256 * (c + 1), :] for n in range(N)], axis=0
        )
        in_maps.append(
            {
                "xTc": cast(xc.T),
                "wqT": wqT,
                "wkT": wkT,
                "wvT": wvT,
                "woT": woT,
            }
        )
    return in_maps


def kernel(x, Wq, Wk, Wv, Wo):
    in_maps = make_in_maps({"x": x, "Wq": Wq, "Wk": Wk, "Wv": Wv, "Wo": Wo})
    res = run_bass_kernel_spmd(get_nc(), in_maps, list(range(NC)))
    out = np.empty((N, L, E), dtype=np.float32)
    for c in range(NC):
        o = res.results[c]["outp"].reshape(N, TPN, E)
        out[:, 256 * c : 256 * (c + 1), :] = o
    return out


# revision 24
# speedup vs baseline: 1.4600x; 1.2875x over previous
"""Multi-head attention (N=4, L=2048, E=1024, H=16, DK=64) on 8 TRN2 cores.

The reference splits heads with a PLAIN RESHAPE (n, l, H*DK) -> (n, H, l, DK),
so "head" h is really a contiguous block of 128 tokens, and the 2048 attention
positions inside it are (token, s) pairs where s indexes sixteen 64-wide
E-slices.  Per (batch, block):
    Qb = q[n, 128b:128b+128, :].reshape(2048, 64)   (same for K, V)
    out_block = softmax(Qb Kb^T / 8) Vb  -> reshape(128, E) -> rows of out
Positions are processed in permuted order p' = 128*s + tok (a permutation of
the softmax axis; unpermuted on the way out).

Sharding: core c owns token rows [n, 256c : 256c+256) for every batch n (two
128-token blocks per batch).  Outputs are disjoint rows; the host scatters.
Each core gets the full weights (streamed in halves) and only its x columns.

Matmuls in bf16 (fp32 PSUM accumulate).  Per-core flow:
  x_sb [E, 1024 tok] resident.
  V:    V_nat [128 tok, E] per (n, B), evicted into per-s slices + ones col.
  Q/K:  [e_out 128, tok 512 (2 batches)] PSUM tiles evicted straight into the
        permuted layout q1t/k1t [128 = 2B x 64 d, n, 2048 p'].
  Attention per (n, u = q' chunk of 512): 8 key-tile pairs; scores for both
        blocks row-packed on the PE (disjoint 64-row groups), written as BF16
        psum; exp on ScalarE over [128, 2048] (scale=1/8 folded), bf16 out;
        PV accumulates [V|ones].T @ expS -> [65, 512] fp32 (row 64 = denom);
        rows 0-63 evicted unnormalized into opT, denom row collected.
  Normalize per batch: one batched reciprocal [8, 512] on DVE, GPSIMD
        partition_broadcast, in-place DVE multiply on opT.
  Out proj per (n, B): accumulate over 8 e_in tiles vs woT halves, DMA out.
"""

import ml_dtypes
import numpy as np

import concourse.bass as bass
import concourse.mybir as mybir
import concourse.tile as tile
from concourse import bacc
from concourse.bass_utils import run_bass_kernel_spmd

N, L, E, H = 4, 2048, 1024, 16
DK = E // H  # 64
NC = 8
BPC = 2  # token blocks per core per batch
TPB = 128  # tokens per block
TPN = BPC * TPB  # 256 tokens per batch per core
TC = N * TPN  # 1024 tokens per core
P = 128
QC = 512  # q' chunk
NQC = 2048 // QC  # 4
NKT = 2048 // P  # 16 key tiles (= s values)
ET = E // P  # 8

F32 = mybir.dt.float32
BF16 = mybir.dt.bfloat16
MM_DT = BF16


def build_nc():
    nc = bacc.Bacc("TRN2", target_bir_lowering=False, debug=False, num_devices=NC)

    xTc = nc.dram_tensor("xTc", [E, TC], MM_DT, kind="ExternalInput").ap()
    wqT = nc.dram_tensor("wqT", [E, E], MM_DT, kind="ExternalInput").ap()
    wkT = nc.dram_tensor("wkT", [E, E], MM_DT, kind="ExternalInput").ap()
    wvT = nc.dram_tensor("wvT", [E, E], MM_DT, kind="ExternalInput").ap()
    woT = nc.dram_tensor("woT", [E, E], MM_DT, kind="ExternalInput").ap()
    outp = nc.dram_tensor("outp", [TC, E], F32, kind="ExternalOutput").ap()

    with tile.TileContext(nc) as tc:
        with (
            tc.tile_pool(name="const", bufs=1) as const,
            tc.tile_pool(name="wpool", bufs=2) as wpool,
            tc.tile_pool(name="xv", bufs=2) as xv_pool,
            tc.tile_pool(name="qk1", bufs=1) as qk1_pool,
            tc.tile_pool(name="expp", bufs=4) as exp_pool,
            tc.tile_pool(name="opt", bufs=2) as opt_pool,
            tc.tile_pool(name="nrm", bufs=2) as nrm_pool,
            tc.tile_pool(name="ops", bufs=2) as op_pool,
            tc.tile_pool(name="scps", bufs=2, space="PSUM") as sc_psum,
            tc.tile_pool(name="pvps", bufs=2, space="PSUM") as pv_psum,
            tc.tile_pool(name="auxps", bufs=2, space="PSUM") as aux_psum,
        ):
            ones_f32 = const.tile([P, P], F32)
            nc.vector.memset(ones_f32[:], 1.0)
            ones_r = const.tile([P, P], mybir.dt.float32r)
            nc.vector.tensor_copy(ones_r[:], ones_f32[:])

            # ---- resident x ----
            x_sb = xv_pool.tile([P, ET, TC], MM_DT, tag="xv", name="x_sb")
            nc.sync.dma_start(out=x_sb[:], in_=xTc.rearrange("(a p) t -> p a t", p=P))

            def load_w_half(w_dram, half, nm):
                w_sb = wpool.tile([P, ET, E // 2], MM_DT, tag="w", name=nm)
                src = w_dram[:, half * (E // 2) : (half + 1) * (E // 2)]
                nc.sync.dma_start(
                    out=w_sb[:], in_=src.rearrange("(a p) d -> p a d", p=P)
                )
                return w_sb

            # ---- V phase ----
            v_sb = xv_pool.tile([P, N, BPC, NKT, DK + 1], MM_DT, tag="xv", name="v_sb")
            for half in range(2):
                wv_sb = load_w_half(wvT, half, f"wv{half}")
                for n in range(N):
                    for B in range(BPC):
                        tok0 = n * TPN + B * TPB
                        ps = aux_psum.tile([P, 512], F32, tag="aux", name="vps")
                        for a in range(ET):
                            nc.tensor.matmul(
                                ps[:],
                                x_sb[:, a, tok0 : tok0 + TPB],
                                wv_sb[:, a, :],
                                start=(a == 0),
                                stop=(a == ET - 1),
                            )
                        # all eight 64-wide s-slices in one strided copy
                        nc.vector.tensor_copy(
                            v_sb[:, n, B, half * 8 : (half + 1) * 8, 0:DK],
                            ps.rearrange("p (s d) -> p s d", d=DK),
                        )
            for n in range(N):
                nc.vector.tensor_copy(
                    v_sb[:, n, :, :, DK], ones_f32[:, 0 : BPC * NKT]
                )

            # ---- Q/K phases: N=512 over two batches, permuted eviction ----
            q1t = qk1_pool.tile([P, N, 2048], MM_DT, tag="q1", name="q1t")
            k1t = qk1_pool.tile([P, N, 2048], MM_DT, tag="k1", name="k1t")
            for w_dram, dst, wnm in ((wqT, q1t, "wq"), (wkT, k1t, "wk")):
                for half in range(2):
                    w_sb = load_w_half(w_dram, half, f"{wnm}{half}")
                    for np2 in range(N // 2):  # batch pair
                        for a2 in range(4):  # e_out tile within half
                            ps = aux_psum.tile([P, 512], F32, tag="aux", name="qkps")
                            for a in range(ET):
                                nc.tensor.matmul(
                                    ps[:],
                                    w_sb[:, a, a2 * P : (a2 + 1) * P],
                                    x_sb[:, a, np2 * 512 : (np2 + 1) * 512],
                                    start=(a == 0),
                                    stop=(a == ET - 1),
                                )
                            for sg in range(2):
                                s = half * 8 + a2 * 2 + sg
                                for B in range(BPC):
                                    # both batches of the pair in one copy
                                    nc.vector.tensor_copy(
                                        dst[
                                            B * DK : (B + 1) * DK,
                                            2 * np2 : 2 * np2 + 2,
                                            s * TPB : (s + 1) * TPB,
                                        ],
                                        ps[
                                            sg * DK : (sg + 1) * DK, :
                                        ].rearrange(
                                            "d (n2 b t) -> d n2 b t", n2=2, b=BPC
                                        )[:, :, B, :],
                                    )

            wo_sb = [load_w_half(woT, half, f"wo{half}") for half in range(2)]

            # ---- attention + normalization + output projection ----
            for n in range(N):
                opT = opt_pool.tile([P, ET, BPC, TPB], MM_DT, tag="opT", name="opT")
                # denominator rows live at 32-aligned partitions of two tiles
                sums = [
                    nrm_pool.tile([P, QC], F32, tag="sums", name=f"sums{_i}")
                    for _i in range(2)
                ]
                for u in range(NQC):
                    qsl = slice(u * QC, (u + 1) * QC)
                    pv = [
                        pv_psum.tile([DK + 1, QC], F32, tag="pv", name=f"pv{_b}")
                        for _b in range(BPC)
                    ]
                    for j in range(NKT):
                        sc = sc_psum.tile([P, BPC, QC], F32, tag="sc")
                        ksl = slice(j * TPB, (j + 1) * TPB)
                        for B in range(BPC):
                            bsl = slice(B * DK, (B + 1) * DK)
                            nc.tensor.matmul(
                                sc[:, B, :],
                                k1t[bsl, n, ksl],
                                q1t[bsl, n, qsl],
                                start=True,
                                stop=True,
                            )
                        exps = exp_pool.tile([P, BPC, QC], MM_DT, tag="exps")
                        nc.scalar.activation(
                            exps[:],
                            sc[:],
                            mybir.ActivationFunctionType.Exp,
                            scale=1.0 / np.sqrt(DK),
                        )
                        for B in range(BPC):
                            nc.tensor.matmul(
                                pv[B][:],
                                v_sb[:, n, B, j, :],
                                exps[:, B, :],
                                start=(j == 0),
                                stop=(j == NKT - 1),
                            )
                    for B in range(BPC):
                        # unnormalized eviction into opT; s = 4u + sp
                        for sg in range(2):
                            nc.vector.tensor_copy(
                                opT[sg * DK : (sg + 1) * DK,
                                    2 * u : 2 * u + 2, B, :],
                                pv[B][0:DK, :].rearrange(
                                    "d (sp t) -> d sp t", t=TPB
                                )[:, sg::2, :],
                            )
                        r_ = B * NQC + u
                        nc.vector.tensor_copy(
                            sums[r_ // 4][32 * (r_ % 4) : 32 * (r_ % 4) + 1, :],
                            pv[B][DK : DK + 1, :],
                        )

                rec = [
                    nrm_pool.tile([P, QC], mybir.dt.float32r, tag="rec",
                                  name=f"rec{_i}")
                    for _i in range(2)
                ]
                with nc.allow_low_precision(reason="softmax denominators"):
                    for _i in range(2):
                        nc.vector.reciprocal(rec[_i][:], sums[_i][:])
                for B in range(BPC):
                    for u in range(NQC):
                        r_ = B * NQC + u
                        rp = 32 * (r_ % 4)
                        bcp = aux_psum.tile([P, QC], F32, tag="aux", name="bcp")
                        nc.tensor.matmul(
                            bcp[:],
                            ones_r[rp : rp + 1, :],
                            rec[r_ // 4][rp : rp + 1, :],
                            start=True,
                            stop=True,
                            tile_position=(rp, 0),
                        )
                        bc = nrm_pool.tile([P, QC], F32, tag="bc", name="bc")
                        nc.vector.tensor_copy(bc[:], bcp[:])
                        for sg in range(2):
                            tgt = opT[sg * DK : (sg + 1) * DK,
                                      2 * u : 2 * u + 2, B, :]
                            nc.vector.tensor_mul(
                                tgt,
                                tgt,
                                bc[sg * DK : (sg + 1) * DK, :].rearrange(
                                    "d (sp t) -> d sp t", t=TPB
                                )[:, sg::2, :],
                            )

                for B in range(BPC):
                    for half in range(2):
                        ps = aux_psum.tile([P, 512], F32, tag="aux", name="opps")
                        for a2 in range(ET):
                            nc.tensor.matmul(
                                ps[:],
                                opT[:, a2, B, :],
                                wo_sb[half][:, a2, :],
                                start=(a2 == 0),
                                stop=(a2 == ET - 1),
                            )
                        op_sb = op_pool.tile([P, 512], F32, tag="op")
                        nc.vector.tensor_copy(op_sb[:], ps[:])
                        r0 = n * TPN + B * TPB
                        nc.sync.dma_start(
                            out=outp[r0 : r0 + TPB, half * 512 : (half + 1) * 512],
                            in_=op_sb[:],
                        )

    nc.compile()
    return nc


_CACHED_NC = None


def get_nc():
    global _CACHED_NC
    if _CACHED_NC is None:
        _CACHED_NC = build_nc()
    return _CACHED_NC


def make_in_maps(inputs):
    x = np.ascontiguousarray(np.asarray(inputs["x"], dtype=np.float32))
    Wq = np.asarray(inputs["Wq"], dtype=np.float32)
    Wk = np.asarray(inputs["Wk"], dtype=np.float32)
    Wv = np.asarray(inputs["Wv"], dtype=np.float32)
    Wo = np.asarray(inputs["Wo"], dtype=np.float32)

    def cast(a):
        return np.ascontiguousarray(a).astype(ml_dtypes.bfloat16)

    wqT = cast(Wq.T)
    wkT = cast(Wk.T)
    wvT = cast(Wv.T)
    woT = cast(Wo.T)
    xr = x.reshape(N, L, E)

    in_maps = []
    for c in range(NC):
        xc = np.concatenate(
            [xr[n, 256 * c : 256 * (c + 1), :] for n in range(N)], axis=0
        )
        in_maps.append(
            {
                "xTc": cast(xc.T),
                "wqT": wqT,
                "wkT": wkT,
                "wvT": wvT,
                "woT": woT,
            }
        )
    return in_maps


def kernel(x, Wq, Wk, Wv, Wo):
    in_maps = make_in_maps({"x": x, "Wq": Wq, "Wk": Wk, "Wv": Wv, "Wo": Wo})
    res = run_bass_kernel_spmd(get_nc(), in_maps, list(range(NC)))
    out = np.empty((N, L, E), dtype=np.float32)
    for c in range(NC):
        o = res.results[c]["outp"].reshape(N, TPN, E)
        out[:, 256 * c : 256 * (c + 1), :] = o
    return out
